# revision 39
# baseline (speedup 1.0000x reference)
"""Trainium2 Bass kernel for ConditionalPositionalEncoding1D-style module:
depthwise conv1d(k=3, pad=1) + BatchNorm1d (inference) + multi-step LIF
(tau=2, v_th=1, hard reset) + residual.

Strategy (8 NeuronCores, data-parallel over batch B=32 -> 4 per core):
  * Slab (chunk-major) layout, fp16: the LIF scan is chunked into K=32
    chunks of L=64 with H=6 halo warm-up steps; slab s holds the
    wavefront column for all 256 (h,b,k) lanes so every DVE access is
    unit-stride and contiguous (strided SBUF reads cost ~2-3x on DVE).
    The host packs x into the slab layout (fp16, halo-duplicated) and
    unpacks the slab-ordered output; all model compute stays on device.
  * conv+BN folded on host into 3 taps + bias. All taps run on TensorE
    as diagonal fp16 matmuls over h-pure slab slices, tap-major in
    PSUM-sized groups; ScalarE drains PSUM->SBUF fp16 adding the bias
    (spline table preloaded; dummy matmuls lift the PE clock gate).
  * LIF: 70 slab steps v' = select(0.5*v + a < 1, u, 0) via a fused
    custom DVE op, two independent h-chains interleaved to hide the
    DVE write-ack latency between dependent steps.
  * phase C: spike = (v'==0) via 4x-mode tensor_scalar into consumed a
    slabs; residual spike+x via identity matmuls accumulating in PSUM
    on the (by then idle) TensorE, ScalarE drains to fp16, chunked
    stores overlap the wave; short final chunk stays on DVE.
"""

import sys

if "/opt/trn_rl_repo" not in sys.path:
    sys.path.insert(0, "/opt/trn_rl_repo")

import numpy as np

import concourse.bass as bass
import concourse.bacc as bacc
import concourse.mybir as mybir
import concourse.tile as tile
import concourse.dve_ops as dve_ops
from concourse.bass_utils import run_bass_kernel_spmd

BN_EPS = 1e-5

# problem geometry (hardcoded per spec)
B, C, T = 32, 256, 2048
NCORES = 8
BP = B // NCORES          # batches per core = 4
P = 128                   # partitions
NH = 2                    # h-structs (channel halves)
L = 64                    # LIF chunk length
H = 6                     # halo steps (sim: +~280 flips vs H=12, within budget)
K = T // L                # chunks per lane = 32
S = L + H                 # wavefront slabs = 76
LN = BP * K               # lanes per slab per h = 128
XS = S + 2                # x slabs (taps need s, s+1, s+2) = 78
OS = S - H                # output slabs = 64

N_WARM = 70               # dummy matmuls to lift the PE clock gate

_lif_op = None


def _get_lif_op():
    """Register the fused LIF-step DVE op (idempotent)."""
    global _lif_op
    if _lif_op is not None:
        return _lif_op
    from concourse.dve_spec import Spec, Src0, Src1, C0, One, Zero, select, lower
    from concourse.dve_uop import DveOpSpec

    u = Src0 * C0 + Src1
    spec = Spec(
        body=select(u < One, u, Zero),
        reference=lambda in0, in1, s0, s1, imm2: (
            lambda u: np.where(u < 1.0, u, 0.0).astype(np.float32)
        )(in0 * s0 + np.asarray(in1).reshape(np.shape(in0))),
    )
    for existing in dve_ops.OPS:
        if existing.name == "LIF_STEP_ANT":
            _lif_op = existing
            return existing
    op = dve_ops.DveOp("LIF_STEP_ANT", spec, subdim=False, uops_sha={})
    dve_ops.OPS.append(op)
    dve_ops._SUB_OPCODE_FOR_NAME[op.name] = (
        dve_ops._CUSTOM_DVE_ROW_BASE + len(dve_ops.OPS) - 1
    )
    dve_ops.CUSTOM_DVE_SPECS[op.name] = op.spec
    for ver in ("v3", "v4"):
        op.uops_sha[ver] = DveOpSpec(
            name=op.name,
            opcode=dve_ops.get_dve_sub_opcode(op.name),
            uops=lower(spec, ver=ver),
            rd1_en=dve_ops.has_src1(spec),
        ).sha(ver)
    _lif_op = op
    return op


def build_program():
    """Build the per-core Bass program (identical on all 8 cores)."""
    lif = _get_lif_op()
    f32 = mybir.dt.float32
    f16 = mybir.dt.float16
    nc = bacc.Bacc(
        "TRN2", target_bir_lowering=False, debug=False, num_devices=NCORES
    )

    W = NH * LN               # interleaved slab width = 256
    x_d = nc.dram_tensor("xw", [P, XS, W], f16, kind="ExternalInput")
    eye_d = nc.dram_tensor("eye", [P, P], f16, kind="ExternalInput")
    wd_d = nc.dram_tensor("wd", [P, NH, 3, P], f16, kind="ExternalInput")
    sv_d = nc.dram_tensor("sv", [P, NH], f32, kind="ExternalInput")
    o_d = nc.dram_tensor("ow", [P, OS, W], f16, kind="ExternalOutput")

    with tile.TileContext(nc) as tc:
        with (
            tc.tile_pool(name="const", bufs=1) as cpool,
            tc.tile_pool(name="xbuf", bufs=1) as xpool,
            tc.tile_pool(name="abuf", bufs=1) as apool,
            tc.tile_pool(name="vbuf", bufs=1) as vpool,
            tc.tile_pool(name="psum", bufs=8, space="PSUM") as ppool,
        ):
            wd_sb = cpool.tile([P, NH, 3, P], f16)
            eye_sb = cpool.tile([P, P], f16)
            sv_sb = cpool.tile([P, NH], f32)
            x_sb = xpool.tile([P, XS, W], f16)
            a_sb = apool.tile([P, S, W], f16)
            v_sb = vpool.tile([P, S, W], f16)
            o_sb = xpool.tile([P, OS, W], f16)
            zeros = cpool.tile([P, W], f16)
            dumw = cpool.tile([P, 16], f16)

            nc.vector.memset(zeros[:], 0.0)
            nc.vector.memset(dumw[:], 0.0)

            # preload the ACT spline table set during the DMA wait
            nc.scalar.activation(
                dumw[:], dumw[:],
                mybir.ActivationFunctionType.Identity, scale=1.0,
            )

            # PE warm-up chatter: lift the HAM clock gate while x streams in
            dps = ppool.tile([P, 16], f32, tag="dps", bufs=1)
            for _ in range(N_WARM):
                nc.tensor.matmul(
                    dps[0:16, :], dumw[:], dumw[:], start=True, stop=True
                )

            # ---- DMA: first x chunk, consts, then the rest of x ----
            nc.sync.dma_start(x_sb[:, 0:6, :], x_d[:, 0:6, :])
            nc.sync.dma_start(wd_sb[:], wd_d[:])
            nc.sync.dma_start(eye_sb[:], eye_d[:])
            nc.sync.dma_start(sv_sb[:], sv_d[:])
            edges = [6, 20, 36, 54, XS]
            for c0, c1 in zip(edges[:-1], edges[1:]):
                nc.sync.dma_start(x_sb[:, c0:c1, :], x_d[:, c0:c1, :])

            # ---- Conv: PE diag matmuls (tap-major groups) + ACT drain.
            #      h-pure [P, n, 128] operands (row stride W) ----
            grp = [0, 4, 8, 16, 32, 48, 64, S]
            for g0, g1 in zip(grp[:-1], grp[1:]):
                for h in range(NH):
                    hs = slice(h * LN, (h + 1) * LN)
                    ntile = (g1 - g0 + 7) // 8
                    pss = []
                    for ti in range(ntile):
                        ps = ppool.tile([P, 1024], f32, name=f"ps{ti}",
                                        tag="ps", bufs=3)
                        pss.append(ps)
                    for tap in range(3):
                        for ti in range(ntile):
                            for half in range(2):
                                s0 = g0 + ti * 8 + half * 4
                                n = min(4, g1 - s0)
                                if n <= 0:
                                    continue
                                nc.tensor.matmul(
                                    pss[ti][:, half * 512 : half * 512 + n * LN],
                                    wd_sb[:, h, tap, :],
                                    x_sb[:, s0 + tap : s0 + tap + n, hs],
                                    start=(tap == 0),
                                    stop=(tap == 2),
                                )
                    for ti in range(ntile):
                        s0 = g0 + ti * 8
                        n = min(8, g1 - s0)
                        nc.scalar.activation(
                            a_sb[:, s0 : s0 + n, hs],
                            pss[ti][:, 0 : n * LN],
                            mybir.ActivationFunctionType.Identity,
                            bias=sv_sb[:, h : h + 1],
                            scale=1.0,
                        )

            # ---- LIF wavefront: S fused DVE steps over contiguous slabs,
            #      phase-C chunks (out = (v==0) + x) interleaved into the
            #      PE-pacing gaps, remainder as tail ----
            def phase_c(c0, c1):
                # spike = (v==0) into the consumed a slabs (4x-capable ts);
                # out = spike + x on the (idle-by-now) PE via identity
                # matmuls accumulating in PSUM; ACT drains to fp16
                nc.vector.tensor_scalar(
                    a_sb[:, c0:c1, :], v_sb[:, c0:c1, :], 0.0, None,
                    mybir.AluOpType.is_equal,
                )
                for t0 in range(c0, c1, 4):
                    n = min(4, c1 - t0)           # slabs per psum tile
                    ps = ppool.tile([P, 1024], f32, name="psc",
                                    tag="ps", bufs=3)
                    for half in range(2):
                        i0 = t0 + half * 2
                        m = min(2, c1 - i0)
                        if m <= 0:
                            continue
                        reg = ps[:, half * 512 : half * 512 + m * W]
                        nc.tensor.matmul(
                            reg, eye_sb[:], a_sb[:, i0 : i0 + m, :],
                            start=True, stop=False,
                        )
                        nc.tensor.matmul(
                            reg, eye_sb[:], x_sb[:, i0 + 1 : i0 + m + 1, :],
                            start=False, stop=True,
                        )
                    nc.scalar.activation(
                        o_sb[:, t0 - H : t0 - H + n, :],
                        ps[:, 0 : n * W],
                        mybir.ActivationFunctionType.Identity,
                        scale=1.0,
                    )
                nc.sync.dma_start(
                    o_d[:, c0 - H : c1 - H, :],
                    o_sb[:, c0 - H : c1 - H, :],
                )

            def phase_c_dve(c0, c1):
                # short-chain tail: ts spike + tt add, both on DVE
                tmp = a_sb[:, c0:c1, :]
                osl = o_sb[:, c0 - H : c1 - H, :]
                nc.vector.tensor_scalar(
                    tmp, v_sb[:, c0:c1, :], 0.0, None,
                    mybir.AluOpType.is_equal,
                )
                nc.vector.tensor_tensor(
                    osl, tmp, x_sb[:, c0 + 1 : c1 + 1, :],
                    mybir.AluOpType.add,
                )
                nc.sync.dma_start(o_d[:, c0 - H : c1 - H, :], osl)

            # dual independent h-chains hide the DVE write-ack latency
            last_c = H
            for s in range(S):
                for h in range(NH):
                    hs = slice(h * LN, (h + 1) * LN)
                    nc.vector._custom_dve(
                        lif,
                        out=v_sb[:, s, hs],
                        in0=zeros[:, hs] if s == 0 else v_sb[:, s - 1, hs],
                        in1=a_sb[:, s, hs],
                        s0=0.5,
                    )
                trig = (s >= 2 * H and (s - 2 * H) % 8 == 0 and s <= 60) \
                    or s == 64
                if trig:
                    phase_c(last_c, s)
                    last_c = s
            phase_c_dve(last_c, S)
    nc.finalize()
    return nc


def _host_constants(conv_w, conv_b, gamma, beta, run_mean, run_var):
    f32 = np.float32
    inv = (np.asarray(gamma, f32)
           / np.sqrt(np.asarray(run_var, f32) + f32(BN_EPS))).astype(f32)
    wt = (np.asarray(conv_w, f32)[:, 0, :] * inv[:, None] * f32(0.5)).astype(f32)
    st = ((np.asarray(conv_b, f32) * inv + np.asarray(beta, f32)
           - np.asarray(run_mean, f32) * inv) * f32(0.5)).astype(f32)
    wd = np.zeros((P, NH, 3, P), np.float16)
    sv = np.zeros((P, NH), f32)
    rng = np.arange(P)
    for h in range(NH):
        for tap in range(3):
            wd[rng, h, tap, rng] = wt[h * P : (h + 1) * P, tap].astype(np.float16)
        sv[:, h] = st[h * P : (h + 1) * P]
    return wd, sv


def _pack_x(xc):
    """[BP, C, T] f32 -> slab-layout [P, XS, NH*LN] fp16 (halo-duplicated)."""
    xh = xc.reshape(BP, NH, P, T).astype(np.float16)
    xp = np.zeros((BP, NH, P, T + H + 2), np.float16)
    xp[..., H + 1 : H + 1 + T] = xh
    idx = L * np.arange(K)[:, None] + np.arange(XS)[None, :]  # [K, XS]
    g = xp[..., idx]                                          # [BP,NH,P,K,XS]
    xw = np.transpose(g, (2, 4, 1, 0, 3))                     # [P,XS,NH,BP,K]
    return np.ascontiguousarray(xw).reshape(P, XS, NH * BP * K)


def _unpack_o(ow):
    """Slab-layout [P, OS, NH*LN] fp16 -> [BP, C, T] f32."""
    o = np.asarray(ow).reshape(P, OS, NH, BP, K)
    o = np.transpose(o, (3, 2, 0, 4, 1))                      # [BP,NH,P,K,OS]
    return np.ascontiguousarray(o).reshape(BP, C, T).astype(np.float32)


def run(inputs, trace=False):
    x = np.asarray(inputs["x"], np.float32)
    wd, sv = _host_constants(
        inputs["conv_w"], inputs["conv_b"], inputs["gamma"],
        inputs["beta"], inputs["run_mean"], inputs["run_var"],
    )
    nc = build_program()
    eye = np.eye(P, dtype=np.float16)
    in_maps = [
        {"xw": _pack_x(x[i * BP : (i + 1) * BP]), "wd": wd, "sv": sv,
         "eye": eye}
        for i in range(NCORES)
    ]
    res = run_bass_kernel_spmd(nc, in_maps, list(range(NCORES)), trace=trace)
    out = np.concatenate(
        [_unpack_o(res.results[i]["ow"]) for i in range(NCORES)], axis=0
    )
    return out, res


def kernel(**inputs):
    out, _ = run(inputs)
    return out


# revision 40
# speedup vs baseline: 1.0332x; 1.0332x over previous
"""Trainium2 Bass kernel for ConditionalPositionalEncoding1D-style module:
depthwise conv1d(k=3, pad=1) + BatchNorm1d (inference) + multi-step LIF
(tau=2, v_th=1, hard reset) + residual.

Strategy (8 NeuronCores, data-parallel over batch B=32 -> 4 per core):
  * Slab (chunk-major) layout, fp16: the LIF scan is chunked into K=32
    chunks of L=64 with H=6 halo warm-up steps; slab s holds the
    wavefront column for all 256 (h,b,k) lanes so every DVE access is
    unit-stride and contiguous (strided SBUF reads cost ~2-3x on DVE).
    The host packs x into the slab layout (fp16, halo-duplicated) and
    unpacks the slab-ordered output; all model compute stays on device.
  * conv+BN folded on host into 3 taps + bias. All taps run on TensorE
    as diagonal fp16 matmuls over h-pure slab slices, tap-major in
    PSUM-sized groups; ScalarE drains PSUM->SBUF fp16 adding the bias
    (spline table preloaded; dummy matmuls lift the PE clock gate).
  * LIF: 70 slab steps v' = select(0.5*v + a < 1, u, 0) via a fused
    custom DVE op, two independent h-chains interleaved to hide the
    DVE write-ack latency between dependent steps.
  * phase C: spike = (v'==0) via 4x-mode tensor_scalar into consumed a
    slabs; residual spike+x via identity matmuls accumulating in PSUM
    on the (by then idle) TensorE, ScalarE drains to fp16, chunked
    stores overlap the wave; short final chunk stays on DVE.
"""

import sys

if "/opt/trn_rl_repo" not in sys.path:
    sys.path.insert(0, "/opt/trn_rl_repo")

import numpy as np

import concourse.bass as bass
import concourse.bacc as bacc
import concourse.mybir as mybir
import concourse.tile as tile
import concourse.dve_ops as dve_ops
from concourse.bass_utils import run_bass_kernel_spmd

BN_EPS = 1e-5

# problem geometry (hardcoded per spec)
B, C, T = 32, 256, 2048
NCORES = 8
BP = B // NCORES          # batches per core = 4
P = 128                   # partitions
NH = 2                    # h-structs (channel halves)
L = 64                    # LIF chunk length
H = 6                     # halo steps (sim: +~280 flips vs H=12, within budget)
K = T // L                # chunks per lane = 32
S = L + H                 # wavefront slabs = 76
LN = BP * K               # lanes per slab per h = 128
XS = S + 2                # x slabs (taps need s, s+1, s+2) = 78
OS = S - H                # output slabs = 64

N_WARM = 70               # dummy matmuls to lift the PE clock gate

_lif_op = None


def _get_lif_op():
    """Register the fused LIF-step DVE op (idempotent)."""
    global _lif_op
    if _lif_op is not None:
        return _lif_op
    from concourse.dve_spec import Spec, Src0, Src1, C0, One, Zero, select, lower
    from concourse.dve_uop import DveOpSpec

    u = Src0 * C0 + Src1
    spec = Spec(
        body=select(u < One, u, Zero),
        reference=lambda in0, in1, s0, s1, imm2: (
            lambda u: np.where(u < 1.0, u, 0.0).astype(np.float32)
        )(in0 * s0 + np.asarray(in1).reshape(np.shape(in0))),
    )
    for existing in dve_ops.OPS:
        if existing.name == "LIF_STEP_ANT":
            _lif_op = existing
            return existing
    op = dve_ops.DveOp("LIF_STEP_ANT", spec, subdim=False, uops_sha={})
    dve_ops.OPS.append(op)
    dve_ops._SUB_OPCODE_FOR_NAME[op.name] = (
        dve_ops._CUSTOM_DVE_ROW_BASE + len(dve_ops.OPS) - 1
    )
    dve_ops.CUSTOM_DVE_SPECS[op.name] = op.spec
    for ver in ("v3", "v4"):
        op.uops_sha[ver] = DveOpSpec(
            name=op.name,
            opcode=dve_ops.get_dve_sub_opcode(op.name),
            uops=lower(spec, ver=ver),
            rd1_en=dve_ops.has_src1(spec),
        ).sha(ver)
    _lif_op = op
    return op


def build_program():
    """Build the per-core Bass program (identical on all 8 cores)."""
    lif = _get_lif_op()
    f32 = mybir.dt.float32
    f16 = mybir.dt.float16
    nc = bacc.Bacc(
        "TRN2", target_bir_lowering=False, debug=False, num_devices=NCORES
    )

    W = NH * LN               # interleaved slab width = 256
    x_d = nc.dram_tensor("xw", [P, XS, W], f16, kind="ExternalInput")
    eye_d = nc.dram_tensor("eye", [P, P], f16, kind="ExternalInput")
    wd_d = nc.dram_tensor("wd", [P, NH, 3, P], f16, kind="ExternalInput")
    sv_d = nc.dram_tensor("sv", [P, NH], f32, kind="ExternalInput")
    o_d = nc.dram_tensor("ow", [P, OS, W], f16, kind="ExternalOutput")

    with tile.TileContext(nc) as tc:
        with (
            tc.tile_pool(name="const", bufs=1) as cpool,
            tc.tile_pool(name="xbuf", bufs=1) as xpool,
            tc.tile_pool(name="abuf", bufs=1) as apool,
            tc.tile_pool(name="vbuf", bufs=1) as vpool,
            tc.tile_pool(name="psum", bufs=8, space="PSUM") as ppool,
        ):
            wd_sb = cpool.tile([P, NH, 3, P], f16)
            eye_sb = cpool.tile([P, P], f16)
            sv_sb = cpool.tile([P, NH], f32)
            x_sb = xpool.tile([P, XS, W], f16)
            a_sb = apool.tile([P, S, W], f16)
            v_sb = vpool.tile([P, S, W], f16)
            o_sb = xpool.tile([P, OS, W], f16)
            zeros = cpool.tile([P, W], f16)
            dumw = cpool.tile([P, 16], f16)

            nc.vector.memset(zeros[:], 0.0)
            nc.vector.memset(dumw[:], 0.0)

            # preload the ACT spline table set during the DMA wait
            nc.scalar.activation(
                dumw[:], dumw[:],
                mybir.ActivationFunctionType.Identity, scale=1.0,
            )

            # PE warm-up chatter: lift the HAM clock gate while x streams in
            dps = ppool.tile([P, 16], f32, tag="dps", bufs=1)
            for _ in range(N_WARM):
                nc.tensor.matmul(
                    dps[0:16, :], dumw[:], dumw[:], start=True, stop=True
                )

            # ---- DMA: first x chunk, consts, then the rest of x ----
            nc.sync.dma_start(x_sb[:, 0:6, :], x_d[:, 0:6, :])
            nc.sync.dma_start(wd_sb[:], wd_d[:])
            nc.sync.dma_start(eye_sb[:], eye_d[:])
            nc.sync.dma_start(sv_sb[:], sv_d[:])
            edges = [6, 20, 36, 54, XS]
            for c0, c1 in zip(edges[:-1], edges[1:]):
                nc.sync.dma_start(x_sb[:, c0:c1, :], x_d[:, c0:c1, :])

            # ---- Conv: PE diag matmuls (tap-major groups) + ACT drain.
            #      h-pure [P, n, 128] operands (row stride W) ----
            grp = [0, 4, 16, 32, 48, 64, S]
            for g0, g1 in zip(grp[:-1], grp[1:]):
                for h in range(NH):
                    hs = slice(h * LN, (h + 1) * LN)
                    ntile = (g1 - g0 + 7) // 8
                    pss = []
                    for ti in range(ntile):
                        ps = ppool.tile([P, 1024], f32, name=f"ps{ti}",
                                        tag="ps", bufs=3)
                        pss.append(ps)
                    for tap in range(3):
                        for ti in range(ntile):
                            for half in range(2):
                                s0 = g0 + ti * 8 + half * 4
                                n = min(4, g1 - s0)
                                if n <= 0:
                                    continue
                                nc.tensor.matmul(
                                    pss[ti][:, half * 512 : half * 512 + n * LN],
                                    wd_sb[:, h, tap, :],
                                    x_sb[:, s0 + tap : s0 + tap + n, hs],
                                    start=(tap == 0),
                                    stop=(tap == 2),
                                )
                    for ti in range(ntile):
                        s0 = g0 + ti * 8
                        n = min(8, g1 - s0)
                        nc.scalar.activation(
                            a_sb[:, s0 : s0 + n, hs],
                            pss[ti][:, 0 : n * LN],
                            mybir.ActivationFunctionType.Identity,
                            bias=sv_sb[:, h : h + 1],
                            scale=1.0,
                        )

            # ---- LIF wavefront: S fused DVE steps over contiguous slabs,
            #      phase-C chunks (out = (v==0) + x) interleaved into the
            #      PE-pacing gaps, remainder as tail ----
            def phase_c(c0, c1):
                # spike = (v==0) into the consumed a slabs (4x-capable ts);
                # out = spike + x on the (idle-by-now) PE via identity
                # matmuls accumulating in PSUM; ACT drains to fp16
                nc.vector.tensor_scalar(
                    a_sb[:, c0:c1, :], v_sb[:, c0:c1, :], 0.0, None,
                    mybir.AluOpType.is_equal,
                )
                for t0 in range(c0, c1, 4):
                    n = min(4, c1 - t0)           # slabs per psum tile
                    ps = ppool.tile([P, 1024], f32, name="psc",
                                    tag="ps", bufs=3)
                    for half in range(2):
                        i0 = t0 + half * 2
                        m = min(2, c1 - i0)
                        if m <= 0:
                            continue
                        reg = ps[:, half * 512 : half * 512 + m * W]
                        nc.tensor.matmul(
                            reg, eye_sb[:], a_sb[:, i0 : i0 + m, :],
                            start=True, stop=False,
                        )
                        nc.tensor.matmul(
                            reg, eye_sb[:], x_sb[:, i0 + 1 : i0 + m + 1, :],
                            start=False, stop=True,
                        )
                    nc.scalar.activation(
                        o_sb[:, t0 - H : t0 - H + n, :],
                        ps[:, 0 : n * W],
                        mybir.ActivationFunctionType.Identity,
                        scale=1.0,
                    )
                nc.sync.dma_start(
                    o_d[:, c0 - H : c1 - H, :],
                    o_sb[:, c0 - H : c1 - H, :],
                )

            def phase_c_dve(c0, c1):
                # short-chain tail: ts spike + tt add, both on DVE
                tmp = a_sb[:, c0:c1, :]
                osl = o_sb[:, c0 - H : c1 - H, :]
                nc.vector.tensor_scalar(
                    tmp, v_sb[:, c0:c1, :], 0.0, None,
                    mybir.AluOpType.is_equal,
                )
                nc.vector.tensor_tensor(
                    osl, tmp, x_sb[:, c0 + 1 : c1 + 1, :],
                    mybir.AluOpType.add,
                )
                nc.sync.dma_start(o_d[:, c0 - H : c1 - H, :], osl)

            # dual independent h-chains hide the DVE write-ack latency
            last_c = H
            for s in range(S):
                for h in range(NH):
                    hs = slice(h * LN, (h + 1) * LN)
                    nc.vector._custom_dve(
                        lif,
                        out=v_sb[:, s, hs],
                        in0=zeros[:, hs] if s == 0 else v_sb[:, s - 1, hs],
                        in1=a_sb[:, s, hs],
                        s0=0.5,
                    )
                trig = (s >= 2 * H and (s - 2 * H) % 8 == 0 and s <= 60) \
                    or s == 64
                if trig:
                    phase_c(last_c, s)
                    last_c = s
            phase_c_dve(last_c, S)
    nc.finalize()
    return nc


def _host_constants(conv_w, conv_b, gamma, beta, run_mean, run_var):
    f32 = np.float32
    inv = (np.asarray(gamma, f32)
           / np.sqrt(np.asarray(run_var, f32) + f32(BN_EPS))).astype(f32)
    wt = (np.asarray(conv_w, f32)[:, 0, :] * inv[:, None] * f32(0.5)).astype(f32)
    st = ((np.asarray(conv_b, f32) * inv + np.asarray(beta, f32)
           - np.asarray(run_mean, f32) * inv) * f32(0.5)).astype(f32)
    wd = np.zeros((P, NH, 3, P), np.float16)
    sv = np.zeros((P, NH), f32)
    rng = np.arange(P)
    for h in range(NH):
        for tap in range(3):
            wd[rng, h, tap, rng] = wt[h * P : (h + 1) * P, tap].astype(np.float16)
        sv[:, h] = st[h * P : (h + 1) * P]
    return wd, sv


def _pack_x(xc):
    """[BP, C, T] f32 -> slab-layout [P, XS, NH*LN] fp16 (halo-duplicated)."""
    xh = xc.reshape(BP, NH, P, T).astype(np.float16)
    xp = np.zeros((BP, NH, P, T + H + 2), np.float16)
    xp[..., H + 1 : H + 1 + T] = xh
    idx = L * np.arange(K)[:, None] + np.arange(XS)[None, :]  # [K, XS]
    g = xp[..., idx]                                          # [BP,NH,P,K,XS]
    xw = np.transpose(g, (2, 4, 1, 0, 3))                     # [P,XS,NH,BP,K]
    return np.ascontiguousarray(xw).reshape(P, XS, NH * BP * K)


def _unpack_o(ow):
    """Slab-layout [P, OS, NH*LN] fp16 -> [BP, C, T] f32."""
    o = np.asarray(ow).reshape(P, OS, NH, BP, K)
    o = np.transpose(o, (3, 2, 0, 4, 1))                      # [BP,NH,P,K,OS]
    return np.ascontiguousarray(o).reshape(BP, C, T).astype(np.float32)


def run(inputs, trace=False):
    x = np.asarray(inputs["x"], np.float32)
    wd, sv = _host_constants(
        inputs["conv_w"], inputs["conv_b"], inputs["gamma"],
        inputs["beta"], inputs["run_mean"], inputs["run_var"],
    )
    nc = build_program()
    eye = np.eye(P, dtype=np.float16)
    in_maps = [
        {"xw": _pack_x(x[i * BP : (i + 1) * BP]), "wd": wd, "sv": sv,
         "eye": eye}
        for i in range(NCORES)
    ]
    res = run_bass_kernel_spmd(nc, in_maps, list(range(NCORES)), trace=trace)
    out = np.concatenate(
        [_unpack_o(res.results[i]["ow"]) for i in range(NCORES)], axis=0
    )
    return out, res


def kernel(**inputs):
    out, _ = run(inputs)
    return out
